# revision 1
# baseline (speedup 1.0000x reference)
"""Trainium2 Bass kernel for nn_SubspaceLinopFactory (subspace NUDFT forward op).

Math (reference):
  s[a,c,h,w] = x[a,h,w] * mps[c,h,w]
  E[r,k,(h,w)] = exp(-i*(trj[r,0,k]*gy[h] + trj[r,1,k]*gx[w]))   (separable)
  y[a,r,c,k] = sum_hw E * s
  z[r,t,c,k] = sum_a phi[a,t] * y[a,r,c,k] * sqrt_dcf[r,k]
  out[t,c,k] = z[subsamp_idx[t], t, c, k]

Sharding: trajectory r -> core r (R == 8 == n_cores). Each core computes
z[t,c,k] for all t with its own r; host gathers rows where subsamp_idx[t]==r.

Device pipeline per core (separable NUDFT, fp16 matmul operands / f32 accum):
  - trig tables per k-chunk: host stages packed phase inputs in "turns"
    ([sin|cos] halves; the cos half pre-shifted by a quarter turn), ScalarE
    Copy applies the per-partition gy/2pi scale, VectorE int32-cast roundtrip
    gives frac = m-round(m) in [-.5,.5], ScalarE Sin(2pi*frac) -> fp16 tables.
  - stage 1 (TensorE, fp16): P[(a,c,h),k] = sum_w sT[w,ach]*(dcf*cos_x)[w,k],
    Q likewise with sin_x. 6 m-tiles x 512-wide k-chunks, PSUM f32.
  - ScalarE casts P,Q PSUM->SBUF fp16; VectorE products A=cy*P, B=sy*Q,
    C=cy*Q, D=sy*P (fp16 2x mode).
  - h-reduction (TensorE): +-1 selector matmuls contract (ac,h) partitions:
    y_re[ac,k] = sum_h A-B, y_im = -(C+D), PSUM-accumulated over m-tiles.
  - phi expansion (TensorE): z[(t,c),k] = phiT.T @ y  (rows = t*4+c = 128).
  - z_re, z_im [128,1024] f32 -> host gathers into [T,C,K] complex64.
"""
import numpy as np

A, T, C, R, D, K, H, W = 3, 32, 4, 8, 2, 1024, 64, 64
N_CORES = 8
ACH = A * C * H          # 768
MT = ACH // 128          # 6 m-tiles
KC = 512                 # k-chunk (one PSUM bank of f32)
NKC = K // KC            # 2

_CACHE = {}


def _build_nc():
    import concourse.bacc as bacc
    import concourse.tile as tile
    import concourse.mybir as mybir

    AF = mybir.ActivationFunctionType
    OP = mybir.AluOpType
    F32 = mybir.dt.float32
    F16 = mybir.dt.float16
    I32 = mybir.dt.int32
    TWO_PI = float(2 * np.pi)

    nc = bacc.Bacc(None, target_bir_lowering=False)

    # batched inputs: big64 = [txr2 | dcf2 | xr | mr] on 64 partitions,
    # big128 = [tyr2 | pp] on 128, sel = [selp | selm] fp16, phit fp16.
    W64 = 2 * K + 2 * K + ACH + ACH  # 5632
    d_b64 = nc.dram_tensor("b64", [64, W64], F32, kind="ExternalInput")
    d_b128 = nc.dram_tensor("b128", [128, 2 * K + 2], F32, kind="ExternalInput")
    d_sel = nc.dram_tensor("sel", [128, 24 * MT], F16, kind="ExternalInput")
    d_phit = nc.dram_tensor("phit", [12, 128], F16, kind="ExternalInput")
    d_zre = nc.dram_tensor("zre", [128, K], F32, kind="ExternalOutput")
    d_zim = nc.dram_tensor("zim", [128, K], F32, kind="ExternalOutput")

    with tile.TileContext(nc) as tc:
        with (
            tc.tile_pool(name="cst", bufs=1) as cst,
            tc.tile_pool(name="tabw", bufs=2) as tabw,
            tc.tile_pool(name="tbl", bufs=2) as tblp,
            tc.tile_pool(name="work", bufs=3) as work,
            tc.tile_pool(name="psA", bufs=2, space="PSUM") as psA,
            tc.tile_pool(name="psY", bufs=1, space="PSUM") as psY,
            tc.tile_pool(name="psZ", bufs=1, space="PSUM") as psZ,
        ):
            b64 = cst.tile([64, W64], F32)
            b128 = cst.tile([128, 2 * K + 2], F32)
            sel = cst.tile([128, 24 * MT], F16)
            phit = cst.tile([12, 128], F16)
            nc.sync.dma_start(b64[:], d_b64[:])
            nc.sync.dma_start(b128[:], d_b128[:])
            nc.sync.dma_start(sel[:], d_sel[:])
            nc.sync.dma_start(phit[:], d_phit[:])

            txr2 = b64[:, 0:2 * K].rearrange("p (s k) -> p s k", s=2)
            dcf2 = b64[:, 2 * K:4 * K].rearrange("p (s k) -> p s k", s=2)
            xr = b64[:, 4 * K:4 * K + ACH]
            mr = b64[:, 4 * K + ACH:4 * K + 2 * ACH]
            tyr2 = b128[:, 0:2 * K].rearrange("p (s k) -> p s k", s=2)
            ppy = b128[:, 2 * K:2 * K + 1]
            ppx = b128[:64, 2 * K + 1:2 * K + 2]

            # sT = x_rep * mps_rep  -> fp16 [64, ACH]
            sT = cst.tile([64, ACH], F16)
            nc.vector.tensor_tensor(sT[:], xr[:], mr[:], OP.mult)

            selp = sel[:, 0:12 * MT]
            selm = sel[:, 12 * MT:24 * MT]

            zout_re = cst.tile([128, K], F32)
            zout_im = cst.tile([128, K], F32)

            def trig_chunk(src, scale_ap, P, kc, name, out_dt):
                """[P, 2, KC] fp16 table chunk: [:,0,:]=sin, [:,1,:]=cos."""
                ks = slice(kc * KC, (kc + 1) * KC)
                m = tabw.tile([P, 2, KC], F32, tag=f"m{name}")
                nc.scalar.activation(m[:], src[:, :, ks], AF.Copy, scale=scale_ap)
                mi = tabw.tile([P, 2, KC], I32, tag=f"mi{name}")
                nc.vector.tensor_copy(mi[:], m[:])
                mf = tabw.tile([P, 2, KC], F32, tag=f"mf{name}")
                nc.vector.tensor_copy(mf[:], mi[:])
                fr = tabw.tile([P, 2, KC], F32, tag=f"fr{name}")
                nc.vector.tensor_tensor(fr[:], m[:], mf[:], OP.subtract)
                o = tblp.tile([P, 2, KC], out_dt, tag=f"tbl{name}")
                nc.scalar.activation(o[:], fr[:], AF.Sin, scale=TWO_PI)
                return o

            for kc in range(NKC):
                ks = slice(kc * KC, (kc + 1) * KC)
                xt = trig_chunk(txr2, ppx, 64, kc, "x", F32)
                xtd = tblp.tile([64, 2, KC], F16, tag="xtd")
                nc.vector.tensor_tensor(xtd[:], xt[:], dcf2[:, :, ks], OP.mult)
                yt = trig_chunk(tyr2, ppy, 128, kc, "y", F16)

                yre = psY.tile([12, KC], F32, tag="yre")
                yim = psY.tile([12, KC], F32, tag="yim")
                for j in range(MT):
                    js = slice(j * 128, (j + 1) * 128)
                    p_ps = psA.tile([128, KC], F32, tag="p")
                    q_ps = psA.tile([128, KC], F32, tag="q")
                    nc.tensor.matmul(p_ps[:], sT[:, js], xtd[:, 1, :],
                                     start=True, stop=True)
                    nc.tensor.matmul(q_ps[:], sT[:, js], xtd[:, 0, :],
                                     start=True, stop=True)
                    pc = work.tile([128, KC], F16, tag="pc")
                    qc = work.tile([128, KC], F16, tag="qc")
                    nc.scalar.copy(pc[:], p_ps[:])
                    nc.scalar.copy(qc[:], q_ps[:])
                    prodA = work.tile([128, KC], F16, tag="A")
                    prodB = work.tile([128, KC], F16, tag="B")
                    prodC = work.tile([128, KC], F16, tag="C")
                    prodD = work.tile([128, KC], F16, tag="D")
                    nc.vector.tensor_tensor(prodA[:], pc[:], yt[:, 1, :], OP.mult)
                    nc.vector.tensor_tensor(prodB[:], qc[:], yt[:, 0, :], OP.mult)
                    nc.vector.tensor_tensor(prodC[:], qc[:], yt[:, 1, :], OP.mult)
                    nc.vector.tensor_tensor(prodD[:], pc[:], yt[:, 0, :], OP.mult)
                    sj = slice(j * 12, (j + 1) * 12)
                    nc.tensor.matmul(yre[:], selp[:, sj], prodA[:],
                                     start=(j == 0), stop=False,
                                     skip_group_check=True)
                    nc.tensor.matmul(yre[:], selm[:, sj], prodB[:],
                                     start=False, stop=(j == MT - 1),
                                     skip_group_check=True)
                    nc.tensor.matmul(yim[:], selm[:, sj], prodC[:],
                                     start=(j == 0), stop=False,
                                     skip_group_check=True)
                    nc.tensor.matmul(yim[:], selm[:, sj], prodD[:],
                                     start=False, stop=(j == MT - 1),
                                     skip_group_check=True)
                yre_sb = work.tile([12, KC], F16, tag="yre_sb")
                yim_sb = work.tile([12, KC], F16, tag="yim_sb")
                nc.scalar.copy(yre_sb[:], yre[:])
                nc.scalar.copy(yim_sb[:], yim[:])
                zre_ps = psZ.tile([128, KC], F32, tag="zre")
                zim_ps = psZ.tile([128, KC], F32, tag="zim")
                nc.tensor.matmul(zre_ps[:], phit[:], yre_sb[:], start=True, stop=True)
                nc.tensor.matmul(zim_ps[:], phit[:], yim_sb[:], start=True, stop=True)
                nc.scalar.copy(zout_re[:, ks], zre_ps[:])
                nc.scalar.copy(zout_im[:, ks], zim_ps[:])

            nc.gpsimd.dma_start(d_zre[:], zout_re[:])
            nc.gpsimd.dma_start(d_zim[:], zout_im[:])

    nc.finalize()
    return nc


def _get_nc():
    if "nc" not in _CACHE:
        _CACHE["nc"] = _build_nc()
    return _CACHE["nc"]


def _stage_inputs(x, trj, phi, mps, sqrt_dcf):
    """Per-core input maps. Host staging = layout/replication + tiny
    index/scale constants (phase inputs staged in 'turns' with the cos half
    pre-shifted a quarter turn; gy==0 rows use scale=1 with constant input)."""
    f32, f16 = np.float32, np.float16
    gy = np.arange(H, dtype=np.float64) - H // 2
    inv2pi = 1.0 / (2 * np.pi)

    # per-partition scales (col 0: y for 128 rows; col 1: x for 64 rows)
    sc_y = np.where(gy == 0, 1.0, gy * inv2pi)
    pp = np.zeros((128, 2), np.float64)
    pp[:, 0] = np.concatenate([sc_y, sc_y])
    pp[:64, 1] = sc_y

    # cos-half shift: ty + pi/(2*gy) so m_cos = m_sin + 1/4 turn
    with np.errstate(divide="ignore"):
        shift = np.where(gy == 0, 0.0, np.pi / (2 * gy))

    def packed_phase(tv, P):
        """[P, 2, K]: [:,0,:]=tv (sin), [:,1,:]=tv+shift (cos); gy==0 rows
        get constant 0 / 0.25 (scale is 1 there)."""
        g = np.tile(shift, P // H)
        zero = np.tile(gy == 0, P // H)
        out = np.empty((P, 2, K), np.float64)
        out[:, 0, :] = np.where(zero[:, None], 0.0, tv[None, :])
        out[:, 1, :] = np.where(zero[:, None], 0.25, tv[None, :] + g[:, None])
        return out

    # selectors: block j covers ach rows [j*128,(j+1)*128);
    # partition p -> output column ac = 2*j + p//64
    selp = np.zeros((128, 12 * MT), f16)
    for j in range(MT):
        for p in range(128):
            selp[p, j * 12 + 2 * j + p // 64] = 1.0
    sel = np.concatenate([selp, -selp], axis=1)

    phit = np.zeros((12, 128), f16)
    for a in range(A):
        for c in range(C):
            phit[a * 4 + c, c::4] = phi[a].astype(f16)

    xt = np.ascontiguousarray(x.transpose(2, 0, 1))       # [w, a, h]
    xr = np.broadcast_to(xt[:, :, None, :], (W, A, C, H)).reshape(W, ACH)
    mt = np.ascontiguousarray(mps.transpose(2, 0, 1))     # [w, c, h]
    mr = np.broadcast_to(mt[:, None, :, :], (W, A, C, H)).reshape(W, ACH)

    in_maps = []
    for r in range(N_CORES):
        ty = trj[r, 0, :].astype(np.float64)
        tx = trj[r, 1, :].astype(np.float64)
        b64 = np.empty((64, 5632), f32)
        b64[:, 0:2 * K] = packed_phase(tx, 64).reshape(64, 2 * K)
        b64[:, 2 * K:4 * K] = np.broadcast_to(
            sqrt_dcf[r].astype(f32)[None, None, :], (64, 2, K)).reshape(64, 2 * K)
        b64[:, 4 * K:4 * K + ACH] = xr
        b64[:, 4 * K + ACH:] = mr
        b128 = np.empty((128, 2 * K + 2), f32)
        b128[:, 0:2 * K] = packed_phase(ty, 128).reshape(128, 2 * K)
        b128[:, 2 * K:] = pp
        in_maps.append({"b64": b64, "b128": b128, "sel": sel, "phit": phit})
    return in_maps


def kernel(x, trj, phi, mps, sqrt_dcf, subsamp_idx, _trace=False):
    from concourse.bass_utils import run_bass_kernel_spmd

    nc = _get_nc()
    in_maps = _stage_inputs(np.asarray(x), np.asarray(trj), np.asarray(phi),
                            np.asarray(mps), np.asarray(sqrt_dcf))
    res = run_bass_kernel_spmd(nc, in_maps, core_ids=list(range(N_CORES)),
                               trace=_trace)
    out = np.empty((T, C, K), dtype=np.complex64)
    idx = np.asarray(subsamp_idx).astype(np.int64)
    for t in range(T):
        r = int(idx[t])
        zre = res.results[r]["zre"]
        zim = res.results[r]["zim"]
        for c in range(C):
            out[t, c, :] = zre[t * 4 + c] + 1j * zim[t * 4 + c]
    if _trace:
        kernel._last_results = res
    return out



# revision 2
# speedup vs baseline: 1.2538x; 1.2538x over previous
"""Trainium2 Bass kernel for nn_SubspaceLinopFactory (subspace NUDFT forward).

Math (reference):
  s[a,c,h,w] = x[a,h,w] * mps[c,h,w]
  y[a,c,k]   = sum_hw s * exp(-i*(ty_k*gy_h + tx_k*gx_w))   (separable NUDFT)
  z[t,c,k]   = sum_a phi[a,t] * y[a,c,k] * sqrt_dcf[k],  r = subsamp_idx[t]
Sharding: trajectory r -> core r (R == 8 == n_cores).

Device design (v2, per core):
  gy pairing: gy[h]=h-32; conjugate pairs (+g,-g), g=1..31, halve the
  k-elementwise work; gy=0 joins the plus block; gy=-32 is a small residual
  unit. Host stages paired image columns (spm), phase fractions (range-
  reduced, in turns), dcf, and phi-combined +-selector weights (selphi).
  Per unit u (6 m-tiles of 2 (a,c)-images + 1 residual):
    stage 1 (TensorE, fp16, 64x64 quadrant-tiled): 4 matmuls/chunk write one
      PSUM super [128, 4*512] = [c0:RE|IM, c1:RE|IM]:
        RE = [Pp(plus@cos) rows 0-63 | Qm(minus@sin) rows 64-127]
        IM = [Qp(plus@sin)           | Pm(minus@cos)]
    products (DVE or ScalarE-cast+DVE/GpSimd): prod = ytab*dcf (*) bank
    reduce (TensorE, col-tiled 128x64): z_re += selphi_re.T @ prod_re,
      z_im += selphi_im.T @ prod_im, accumulated in one PSUM super over all
      units (phi + h-reduction fused into one matmul).
  Trig tables on device: ScalarE Sin(2*pi*frac) -> fp16.
  Output: one PSUM->SBUF fp16 copy + DMA of z [128, K]; host scatters rows
  (t-slot, c) into [T, C, K] complex64.
"""
import numpy as np

A, T, C, R, D, K, H, W = 3, 32, 4, 8, 2, 1024, 64, 64
N_CORES = 8
AC = A * C           # 12
MT = AC // 2         # 6 m-tiles
NU = MT + 1          # units incl. residual
NSLOT = 16           # t-slots per launch (M = 4*NSLOT = 64)
KC = 512

# per-unit product mode: 'V' direct DVE from PSUM, 'SV' ScalarE cast + DVE,
# 'SG' ScalarE cast + GpSimd product. Resid is unit 6.
UNIT_MODE = ['V', 'V', 'V', 'V', 'SG', 'SV', 'SV']

_CACHE = {}


def _build_nc():
    import concourse.bacc as bacc
    import concourse.tile as tile
    import concourse.mybir as mybir

    AF = mybir.ActivationFunctionType
    OP = mybir.AluOpType
    F32 = mybir.dt.float32
    F16 = mybir.dt.float16
    TWO_PI = float(2 * np.pi)

    nc = bacc.Bacc(None, target_bir_lowering=False)

    d_xfrac = nc.dram_tensor("xfrac", [128, 2 * K], F16, kind="ExternalInput")
    d_yfrac = nc.dram_tensor("yfrac", [128, 2 * K], F16, kind="ExternalInput")
    d_dcfb = nc.dram_tensor("dcfb", [128, K], F16, kind="ExternalInput")
    d_spm = nc.dram_tensor("spm", [128, 792], F16, kind="ExternalInput")
    d_selphi = nc.dram_tensor("selphi", [128, 14 * 64], F16,
                              kind="ExternalInput")
    d_zout = nc.dram_tensor("zout", [128, K], F16, kind="ExternalOutput")

    with tile.TileContext(nc) as tc:
        with (
            tc.tile_pool(name="cst", bufs=1) as cst,
            tc.tile_pool(name="work", bufs=3) as work,
            tc.tile_pool(name="cwork", bufs=2) as cwork,
            tc.tile_pool(name="psS", bufs=1, space="PSUM") as psS,
            tc.tile_pool(name="psZ", bufs=1, space="PSUM") as psZ,
        ):
            xfrac = cst.tile([128, 2, K], F16)
            yfrac = cst.tile([128, 2, K], F16)
            dcfb = cst.tile([128, K], F16)
            spm = cst.tile([128, 792], F16)
            selphi = cst.tile([128, 14 * 64], F16)
            nc.sync.dma_start(xfrac[:], d_xfrac[:].rearrange(
                "p (s k) -> p s k", s=2))
            nc.sync.dma_start(spm[:], d_spm[:])
            nc.sync.dma_start(yfrac[:], d_yfrac[:].rearrange(
                "p (s k) -> p s k", s=2))
            nc.sync.dma_start(dcfb[:], d_dcfb[:])
            nc.sync.dma_start(selphi[:], d_selphi[:])

            # trig tables: fp16 sin/cos via Sin(2*pi*frac)
            xtab = cst.tile([128, 2, K], F16)   # [:,0,:]=sin_x, [:,1,:]=cos_x
            ytab = cst.tile([128, 2, K], F16)   # [:,0,:]=main, [:,1,:]=resid
            for kc in range(2):
                ks = slice(kc * KC, (kc + 1) * KC)
                nc.scalar.activation(xtab[:, :, ks], xfrac[:, :, ks],
                                     AF.Sin, scale=TWO_PI)
            nc.scalar.activation(ytab[:, 0, :], yfrac[:, 0, :],
                                 AF.Sin, scale=TWO_PI)
            nc.scalar.activation(ytab[:, 1, :], yfrac[:, 1, :],
                                 AF.Sin, scale=TWO_PI)

            # dcf-premultiplied product tables
            # ytmd [128, 4, KC] = [Md-c0, Md-c0, Md-c1, Md-c1]
            ytmd = cst.tile([128, 4, KC], F16)
            ytrd = cst.tile([128, K], F16)
            for kc in range(2):
                ks = slice(kc * KC, (kc + 1) * KC)
                for half in range(2):
                    nc.vector.tensor_tensor(ytmd[:, 2 * kc + half, :],
                                            ytab[:, 0, ks], dcfb[:, ks],
                                            OP.mult)
            nc.vector.tensor_tensor(ytrd[:], ytab[:, 1, :], dcfb[:], OP.mult)

            z = psZ.tile([128, K], F32)       # rows 0-63 z_re, 64-127 z_im
            zout_sb = cst.tile([128, K], F16)

            for u in range(NU):
                mode = UNIT_MODE[u]
                nck = 4 if u < MT else 2
                # PSUM super: m-tiles [c0RE, c0IM, c1RE, c1IM]; resid [c0, c1]
                bank = psS.tile([128, nck, KC], F32, tag=f"bank{nck}")
                for kc in range(2):
                    ks = slice(kc * KC, (kc + 1) * KC)
                    if u < MT:
                        cb = u * 128
                        # 4 quadrant matmuls (auto tile_position from bases)
                        nc.tensor.matmul(bank[0:64, 2 * kc, :],
                                         spm[0:64, cb:cb + 64],
                                         xtab[0:64, 1, ks],
                                         start=True, stop=True)          # Pp
                        nc.tensor.matmul(bank[64:128, 2 * kc, :],
                                         spm[64:128, cb + 64:cb + 128],
                                         xtab[64:128, 0, ks],
                                         start=True, stop=True)          # Qm
                        nc.tensor.matmul(bank[0:64, 2 * kc + 1, :],
                                         spm[64:128, cb:cb + 64],
                                         xtab[64:128, 0, ks],
                                         start=True, stop=True)          # Qp
                        nc.tensor.matmul(bank[64:128, 2 * kc + 1, :],
                                         spm[0:64, cb + 64:cb + 128],
                                         xtab[0:64, 1, ks],
                                         start=True, stop=True)          # Pm
                    else:
                        nc.tensor.matmul(bank[0:24, kc, :],
                                         spm[0:64, 768:792],
                                         xtab[0:64, 1, ks],
                                         start=True, stop=True)     # P0 rows
                        nc.tensor.matmul(bank[64:88, kc, :],
                                         spm[64:128, 768:792],
                                         xtab[64:128, 0, ks],
                                         start=True, stop=True)     # Q0 rows

                for kc in range(2):
                    ks = slice(kc * KC, (kc + 1) * KC)
                    ncols = 2 if u < MT else 1
                    fd = ncols * KC
                    if u < MT:
                        bsl = bank[:, 2 * kc:2 * kc + 2, :]
                        tsl = ytmd[:, 0:2, :] if kc == 0 else ytmd[:, 2:4, :]
                    else:
                        bsl = bank[:, kc, :]
                        tsl = ytrd[:, ks]
                    if mode == 'V':
                        prod = work.tile([128, 2, KC], F16, tag="prod")
                        psl = prod[:, 0:ncols, :] if ncols == 2 else prod[:, 0, :]
                        nc.vector.tensor_tensor(psl, bsl, tsl, OP.mult)
                    else:
                        cast = cwork.tile([128, 2, KC], F16, tag="cast")
                        csl = cast[:, 0:ncols, :] if ncols == 2 else cast[:, 0, :]
                        nc.scalar.copy(csl, bsl)
                        prod = work.tile([128, 2, KC], F16, tag="prod")
                        psl = prod[:, 0:ncols, :] if ncols == 2 else prod[:, 0, :]
                        if mode == 'SV':
                            nc.vector.tensor_tensor(psl, csl, tsl, OP.mult)
                        else:
                            nc.gpsimd.tensor_tensor(psl, csl, tsl, OP.mult)

                    # fused h-reduce + phi matmuls (col-tiled pair)
                    st = (u == 0)
                    sp = (u == NU - 1)
                    re_rhs = prod[:, 0, :]
                    im_rhs = prod[:, 1, :] if u < MT else prod[:, 0, :]
                    nc.tensor.matmul(z[0:64, ks],
                                     selphi[:, (2 * u) * 64:(2 * u + 1) * 64],
                                     re_rhs, start=st, stop=sp,
                                     skip_group_check=True)
                    nc.tensor.matmul(z[64:128, ks],
                                     selphi[:, (2 * u + 1) * 64:(2 * u + 2) * 64],
                                     im_rhs, start=st, stop=sp,
                                     skip_group_check=True)

            nc.scalar.copy(zout_sb[:], z[:])
            nc.gpsimd.dma_start(d_zout[:], zout_sb[:])

    nc.finalize()
    return nc


def _get_nc():
    if "nc" not in _CACHE:
        _CACHE["nc"] = _build_nc()
    return _CACHE["nc"]


def _stage_core(r, x, trj, phi, mps, sqrt_dcf, owned_ts):
    """Host staging for core r: layout/pairing of inputs, phase fractions
    (range-reduced phases in turns), and phi-signed selector weights."""
    f16 = np.float16
    ty = trj[r, 0, :].astype(np.float64)
    tx = trj[r, 1, :].astype(np.float64)
    inv2pi = 1.0 / (2 * np.pi)

    def frac(v):
        return v - np.round(v)

    gx = (np.arange(W) - W // 2).astype(np.float64)
    mx = np.outer(gx, tx) * inv2pi
    xf = np.empty((128, 2, K), np.float64)
    xf[:64, 0, :] = frac(mx)
    xf[:64, 1, :] = frac(mx + 0.25)
    xf[64:] = xf[:64]

    g = np.arange(32).astype(np.float64)
    my = np.outer(g, ty) * inv2pi
    yf = np.zeros((128, 2, K), np.float64)
    yf[0:32, 0, :] = frac(my + 0.25)
    yf[32:64, 0, :] = yf[0:32, 0, :]
    yf[64:96, 0, :] = frac(my)
    yf[96:128, 0, :] = yf[64:96, 0, :]
    m32 = 32.0 * ty * inv2pi
    yf[0:12, 1, :] = frac(m32 + 0.25)
    yf[12:24, 1, :] = frac(m32)
    yf[64:76, 1, :] = frac(m32)
    yf[76:88, 1, :] = frac(m32 + 0.25)

    dcfb = np.broadcast_to(sqrt_dcf[r].astype(f16), (128, K))

    s = (x[:, None, :, :] * mps[None, :, :, :]).reshape(AC, H, W)
    sp = s[:, 33:64, :]                     # gy = +1..+31
    sm = s[:, 31:0:-1, :]                   # gy = -1..-31
    spl = np.zeros((128, 792), np.float64)
    for j in range(MT):
        for i, ac in enumerate((2 * j, 2 * j + 1)):
            pc = j * 128 + i * 32
            mc = j * 128 + 64 + i * 32
            spl[:64, pc] = s[ac, 32, :]
            spl[:64, pc + 1:pc + 32] = (sp[ac] + sm[ac]).T
            spl[:64, mc + 1:mc + 32] = (sp[ac] - sm[ac]).T
    spl[:64, 768:780] = s[:, 0, :].T
    spl[:64, 780:792] = s[:, 0, :].T
    spl[64:] = spl[:64]

    selphi = np.zeros((128, 14 * 64), np.float64)
    for j in range(MT):
        sre = selphi[:, (2 * j) * 64:(2 * j) * 64 + 64]
        sim = selphi[:, (2 * j + 1) * 64:(2 * j + 1) * 64 + 64]
        for i, ac in enumerate((2 * j, 2 * j + 1)):
            a, c = divmod(ac, C)
            for slot, t in enumerate(owned_ts):
                m = 4 * slot + c
                p = phi[a, t]
                sre[i * 32:(i + 1) * 32, m] = p
                sre[64 + i * 32:96 + i * 32, m] = -p
                sim[i * 32:(i + 1) * 32, m] = -p
                sim[64 + i * 32:96 + i * 32, m] = -p
    sre = selphi[:, 12 * 64:13 * 64]
    sim = selphi[:, 13 * 64:14 * 64]
    for ac in range(AC):
        a, c = divmod(ac, C)
        for slot, t in enumerate(owned_ts):
            m = 4 * slot + c
            p = phi[a, t]
            sre[ac, m] = p
            sre[64 + ac, m] = p
            sim[12 + ac, m] = p
            sim[76 + ac, m] = -p

    return {
        "xfrac": xf.reshape(128, 2 * K).astype(f16),
        "yfrac": yf.reshape(128, 2 * K).astype(f16),
        "dcfb": np.ascontiguousarray(dcfb),
        "spm": spl.astype(f16),
        "selphi": selphi.astype(f16),
    }


def kernel(x, trj, phi, mps, sqrt_dcf, subsamp_idx, _trace=False):
    from concourse.bass_utils import run_bass_kernel_spmd

    x = np.asarray(x, dtype=np.float32)
    trj = np.asarray(trj, dtype=np.float32)
    phi = np.asarray(phi, dtype=np.float32)
    mps = np.asarray(mps, dtype=np.float32)
    sqrt_dcf = np.asarray(sqrt_dcf, dtype=np.float32)
    idx = np.asarray(subsamp_idx).astype(np.int64)

    nc = _get_nc()
    owned = {r: [t for t in range(T) if idx[t] == r] for r in range(R)}
    out = np.empty((T, C, K), dtype=np.complex64)
    launches = max(1, max((len(v) + NSLOT - 1) // NSLOT
                          for v in owned.values()))
    for li in range(launches):
        batch = {r: owned[r][li * NSLOT:(li + 1) * NSLOT] for r in range(R)}
        in_maps = [
            _stage_core(r, x, trj, phi, mps, sqrt_dcf, batch[r])
            for r in range(N_CORES)
        ]
        res = run_bass_kernel_spmd(nc, in_maps, core_ids=list(range(N_CORES)),
                                   trace=_trace)
        for r in range(N_CORES):
            if not batch[r]:
                continue
            zout = res.results[r]["zout"].astype(np.float32)
            for slot, t in enumerate(batch[r]):
                for c in range(C):
                    out[t, c, :] = (zout[4 * slot + c]
                                    + 1j * zout[64 + 4 * slot + c])
        if _trace:
            kernel._last_results = res
    return out


# revision 6
# speedup vs baseline: 1.5211x; 1.2132x over previous
"""Trainium2 Bass kernel for nn_SubspaceLinopFactory (subspace NUDFT forward).

Math (reference):
  s[a,c,h,w] = x[a,h,w] * mps[c,h,w]
  y[a,c,k]   = sum_hw s * exp(-i*(ty_k*gy_h + tx_k*gx_w))   (separable NUDFT)
  z[t,c,k]   = sum_a phi[a,t] * y[a,c,k] * sqrt_dcf[k],  r = subsamp_idx[t]
Sharding: trajectory r -> core r (R == 8 == n_cores).

Device design (v3, per core):
  gy pairing: gy[h]=h-32; conjugate pairs (+g,-g), g=1..31, halve the
  k-elementwise work; gy=0 joins the plus block; gy=-32 is a small residual
  unit. Host stages paired image columns (spm), phase fractions (range-
  reduced phases in turns), dcf, and phi-combined +-selector weights.
  Pipeline per (unit u, k-chunk) with triple-buffered PSUM [128,2,512]:
    stage 1 (TensorE fp16, 64x64 quadrant-tiled, 4 concurrent matmuls):
        RE = [Pp(plus@cos) rows 0-63 | Qm(minus@sin) rows 64-127]
        IM = [Qp(plus@sin)           | Pm(minus@cos)]
    product: prod = (ytab*dcf) (*) bank  -- DVE direct from PSUM, or
        ScalarE fp16 cast + DVE/GpSimd 2x, per-unit mode
    reduce (TensorE, col-tiled 128x64 pair): z_re += selphi_re.T @ prod_re,
        z_im += selphi_im.T @ prod_im  (phi + h-reduction in one matmul,
        accumulated over all units in one PSUM super)
  Trig tables on device: ScalarE Sin(2*pi*frac) -> fp16.
  Output: one PSUM->SBUF fp16 copy + DMA of z [128, K]; host scatters rows
  (t-slot, c) into [T, C, K] complex64.
"""
import numpy as np

A, T, C, R, D, K, H, W = 3, 32, 4, 8, 2, 1024, 64, 64
N_CORES = 8
AC = A * C           # 12
MT = AC // 2         # 6 m-tiles
NU = MT + 1          # units incl. residual
NSLOT = 16           # t-slots per launch (M = 4*NSLOT = 64)
KC = 512

# per-unit product mode: 'V' direct DVE from PSUM, 'SV' ScalarE cast + DVE,
# 'SG' ScalarE cast + GpSimd product. Resid is unit 6.
UNIT_MODE = ['V', 'V', 'V', 'V', 'SV', 'SG', 'SV']

_CACHE = {}


def _build_nc():
    import concourse.bacc as bacc
    import concourse.tile as tile
    import concourse.mybir as mybir

    AF = mybir.ActivationFunctionType
    OP = mybir.AluOpType
    F32 = mybir.dt.float32
    F16 = mybir.dt.float16
    TWO_PI = float(2 * np.pi)

    nc = bacc.Bacc(None, target_bir_lowering=False)

    d_xfrac = nc.dram_tensor("xfrac", [128, 2 * K], F16, kind="ExternalInput")
    d_yfm = nc.dram_tensor("yfm", [128, K], F16, kind="ExternalInput")
    d_yfr = nc.dram_tensor("yfr", [128, K], F16, kind="ExternalInput")
    d_dcfb = nc.dram_tensor("dcfb", [128, K], F16, kind="ExternalInput")
    d_spm = nc.dram_tensor("spm", [128, 792], F16, kind="ExternalInput")
    d_selphi = nc.dram_tensor("selphi", [128, 14 * 64], F16,
                              kind="ExternalInput")
    d_zout = nc.dram_tensor("zout", [128, K], F16, kind="ExternalOutput")

    with tile.TileContext(nc) as tc:
        with (
            tc.tile_pool(name="cst", bufs=1) as cst,
            tc.tile_pool(name="work", bufs=3) as work,
            tc.tile_pool(name="cwork", bufs=2) as cwork,
            tc.tile_pool(name="psS", bufs=3, space="PSUM") as psS,
            tc.tile_pool(name="psZ", bufs=1, space="PSUM") as psZ,
        ):
            # xfrac/xtab chunk-major: [128, chunk, sin|cos, KC]
            xfrac = cst.tile([128, 2, 2, KC], F16)
            yfrac = cst.tile([128, 2, K], F16)
            dcfb = cst.tile([128, K], F16)
            spm = cst.tile([128, 792], F16)
            selphi = cst.tile([128, 14 * 64], F16)
            # spread DMA triggers across the sync and gpsimd queues so the
            # transfers pipeline; xfrac chunk 0 + y-main gate the head.
            nc.sync.dma_start(xfrac[:, 0], d_xfrac[:, 0:K].rearrange(
                "p (s k) -> p s k", s=2))
            nc.gpsimd.dma_start(spm[:], d_spm[:])
            nc.sync.dma_start(yfrac[:, 0, :], d_yfm[:])
            nc.gpsimd.dma_start(xfrac[:, 1], d_xfrac[:, K:2 * K].rearrange(
                "p (s k) -> p s k", s=2))
            nc.sync.dma_start(yfrac[:, 1, :], d_yfr[:])
            nc.gpsimd.dma_start(dcfb[:], d_dcfb[:])
            nc.gpsimd.dma_start(selphi[:], d_selphi[:])

            # trig tables: fp16 sin/cos via Sin(2*pi*frac)
            xtab = cst.tile([128, 2, 2, KC], F16)
            ytab = cst.tile([128, 2, K], F16)   # [:,0,:]=main, [:,1,:]=resid
            nc.scalar.activation(xtab[:, 0], xfrac[:, 0],
                                 AF.Sin, scale=TWO_PI)
            nc.scalar.activation(ytab[:, 0, :], yfrac[:, 0, :],
                                 AF.Sin, scale=TWO_PI)
            nc.scalar.activation(xtab[:, 1], xfrac[:, 1],
                                 AF.Sin, scale=TWO_PI)
            nc.scalar.activation(ytab[:, 1, :], yfrac[:, 1, :],
                                 AF.Sin, scale=TWO_PI)

            # dcf-premultiplied product tables
            # ytmd [128, 4, KC] = [Md-c0, Md-c0, Md-c1, Md-c1]
            ytmd = cst.tile([128, 4, KC], F16)
            ytrd = cst.tile([128, K], F16)
            for kc in range(2):
                ks = slice(kc * KC, (kc + 1) * KC)
                for half in range(2):
                    nc.vector.tensor_tensor(ytmd[:, 2 * kc + half, :],
                                            ytab[:, 0, ks], dcfb[:, ks],
                                            OP.mult)
            nc.vector.tensor_tensor(ytrd[:], ytab[:, 1, :], dcfb[:], OP.mult)

            z = psZ.tile([128, K], F32)       # rows 0-63 z_re, 64-127 z_im
            zout_sb = cst.tile([128, K], F16)

            for u in range(NU):
                mode = UNIT_MODE[u]
                for kc in range(2):
                    ks = slice(kc * KC, (kc + 1) * KC)
                    # PSUM per unit-chunk: [128, RE|IM, KC]
                    bank = psS.tile([128, 2, KC], F32, tag="bank")
                    cosx = xtab[0:64, kc, 1, :]
                    sinx = xtab[64:128, kc, 0, :]
                    if u < MT:
                        cb = u * 128
                        # 4 quadrant matmuls (auto tile_position from bases)
                        nc.tensor.matmul(bank[0:64, 0, :],
                                         spm[0:64, cb:cb + 64], cosx,
                                         start=True, stop=True)          # Pp
                        nc.tensor.matmul(bank[64:128, 0, :],
                                         spm[64:128, cb + 64:cb + 128], sinx,
                                         start=True, stop=True)          # Qm
                        nc.tensor.matmul(bank[0:64, 1, :],
                                         spm[64:128, cb:cb + 64], sinx,
                                         start=True, stop=True)          # Qp
                        nc.tensor.matmul(bank[64:128, 1, :],
                                         spm[0:64, cb + 64:cb + 128], cosx,
                                         start=True, stop=True)          # Pm
                        ncols, tsl = 2, ytmd[:, 2 * kc:2 * kc + 2, :]
                        bsl = bank[:, 0:2, :]
                    else:
                        nc.tensor.matmul(bank[0:24, 0, :],
                                         spm[0:64, 768:792], cosx,
                                         start=True, stop=True)     # P0 rows
                        nc.tensor.matmul(bank[64:88, 0, :],
                                         spm[64:128, 768:792], sinx,
                                         start=True, stop=True)     # Q0 rows
                        ncols, tsl = 1, ytrd[:, ks]
                        bsl = bank[:, 0, :]

                    prod = work.tile([128, 2, KC], F16, tag="prod")
                    psl = prod[:, 0:2, :] if ncols == 2 else prod[:, 0, :]
                    if mode == 'V':
                        nc.vector.tensor_tensor(psl, bsl, tsl, OP.mult)
                    else:
                        cast = cwork.tile([128, 2, KC], F16, tag="cast")
                        csl = cast[:, 0:2, :] if ncols == 2 else cast[:, 0, :]
                        nc.scalar.copy(csl, bsl)
                        if mode == 'SV':
                            nc.vector.tensor_tensor(psl, csl, tsl, OP.mult)
                        else:
                            nc.gpsimd.tensor_tensor(psl, csl, tsl, OP.mult)

                    # fused h-reduce + phi matmuls (col-tiled pair)
                    st = (u == 0)
                    sp = (u == NU - 1)
                    re_rhs = prod[:, 0, :]
                    im_rhs = prod[:, 1, :] if u < MT else prod[:, 0, :]
                    nc.tensor.matmul(z[0:64, ks],
                                     selphi[:, (2 * u) * 64:(2 * u + 1) * 64],
                                     re_rhs, start=st, stop=sp,
                                     skip_group_check=True)
                    nc.tensor.matmul(z[64:128, ks],
                                     selphi[:, (2 * u + 1) * 64:(2 * u + 2) * 64],
                                     im_rhs, start=st, stop=sp,
                                     skip_group_check=True)

            nc.scalar.copy(zout_sb[:], z[:])
            nc.gpsimd.dma_start(d_zout[:], zout_sb[:])

    nc.finalize()
    return nc


def _get_nc():
    if "nc" not in _CACHE:
        _CACHE["nc"] = _build_nc()
    return _CACHE["nc"]


def _stage_core(r, x, trj, phi, mps, sqrt_dcf, owned_ts):
    """Host staging for core r: layout/pairing of inputs, phase fractions
    (range-reduced phases in turns), and phi-signed selector weights."""
    f16 = np.float16
    ty = trj[r, 0, :].astype(np.float64)
    tx = trj[r, 1, :].astype(np.float64)
    inv2pi = 1.0 / (2 * np.pi)

    def frac(v):
        return v - np.round(v)

    gx = (np.arange(W) - W // 2).astype(np.float64)
    mx = np.outer(gx, tx) * inv2pi
    # chunk-major: [128, chunk, sin|cos, KC]
    xf = np.empty((128, 2, 2, KC), np.float64)
    for kc in range(2):
        ks = slice(kc * KC, (kc + 1) * KC)
        xf[:64, kc, 0, :] = frac(mx[:, ks])
        xf[:64, kc, 1, :] = frac(mx[:, ks] + 0.25)
    xf[64:] = xf[:64]

    g = np.arange(32).astype(np.float64)
    my = np.outer(g, ty) * inv2pi
    yf = np.zeros((128, 2, K), np.float64)
    yf[0:32, 0, :] = frac(my + 0.25)
    yf[32:64, 0, :] = yf[0:32, 0, :]
    yf[64:96, 0, :] = frac(my)
    yf[96:128, 0, :] = yf[64:96, 0, :]
    m32 = 32.0 * ty * inv2pi
    yf[0:12, 1, :] = frac(m32 + 0.25)
    yf[12:24, 1, :] = frac(m32)
    yf[64:76, 1, :] = frac(m32)
    yf[76:88, 1, :] = frac(m32 + 0.25)

    dcfb = np.broadcast_to(sqrt_dcf[r].astype(f16), (128, K))

    s = (x[:, None, :, :] * mps[None, :, :, :]).reshape(AC, H, W)
    sp = s[:, 33:64, :]                     # gy = +1..+31
    sm = s[:, 31:0:-1, :]                   # gy = -1..-31
    spl = np.zeros((128, 792), np.float64)
    for j in range(MT):
        for i, ac in enumerate((2 * j, 2 * j + 1)):
            pc = j * 128 + i * 32
            mc = j * 128 + 64 + i * 32
            spl[:64, pc] = s[ac, 32, :]
            spl[:64, pc + 1:pc + 32] = (sp[ac] + sm[ac]).T
            spl[:64, mc + 1:mc + 32] = (sp[ac] - sm[ac]).T
    spl[:64, 768:780] = s[:, 0, :].T
    spl[:64, 780:792] = s[:, 0, :].T
    spl[64:] = spl[:64]

    selphi = np.zeros((128, 14 * 64), np.float64)
    for j in range(MT):
        sre = selphi[:, (2 * j) * 64:(2 * j) * 64 + 64]
        sim = selphi[:, (2 * j + 1) * 64:(2 * j + 1) * 64 + 64]
        for i, ac in enumerate((2 * j, 2 * j + 1)):
            a, c = divmod(ac, C)
            for slot, t in enumerate(owned_ts):
                m = 4 * slot + c
                p = phi[a, t]
                sre[i * 32:(i + 1) * 32, m] = p
                sre[64 + i * 32:96 + i * 32, m] = -p
                sim[i * 32:(i + 1) * 32, m] = -p
                sim[64 + i * 32:96 + i * 32, m] = -p
    sre = selphi[:, 12 * 64:13 * 64]
    sim = selphi[:, 13 * 64:14 * 64]
    for ac in range(AC):
        a, c = divmod(ac, C)
        for slot, t in enumerate(owned_ts):
            m = 4 * slot + c
            p = phi[a, t]
            sre[ac, m] = p
            sre[64 + ac, m] = p
            sim[12 + ac, m] = p
            sim[76 + ac, m] = -p

    return {
        "xfrac": xf.reshape(128, 2 * K).astype(f16),
        "yfm": yf[:, 0, :].astype(f16),
        "yfr": yf[:, 1, :].astype(f16),
        "dcfb": np.ascontiguousarray(dcfb),
        "spm": spl.astype(f16),
        "selphi": selphi.astype(f16),
    }


def kernel(x, trj, phi, mps, sqrt_dcf, subsamp_idx, _trace=False):
    from concourse.bass_utils import run_bass_kernel_spmd

    x = np.asarray(x, dtype=np.float32)
    trj = np.asarray(trj, dtype=np.float32)
    phi = np.asarray(phi, dtype=np.float32)
    mps = np.asarray(mps, dtype=np.float32)
    sqrt_dcf = np.asarray(sqrt_dcf, dtype=np.float32)
    idx = np.asarray(subsamp_idx).astype(np.int64)

    nc = _get_nc()
    owned = {r: [t for t in range(T) if idx[t] == r] for r in range(R)}
    out = np.empty((T, C, K), dtype=np.complex64)
    launches = max(1, max((len(v) + NSLOT - 1) // NSLOT
                          for v in owned.values()))
    for li in range(launches):
        batch = {r: owned[r][li * NSLOT:(li + 1) * NSLOT] for r in range(R)}
        in_maps = [
            _stage_core(r, x, trj, phi, mps, sqrt_dcf, batch[r])
            for r in range(N_CORES)
        ]
        res = run_bass_kernel_spmd(nc, in_maps, core_ids=list(range(N_CORES)),
                                   trace=_trace)
        for r in range(N_CORES):
            if not batch[r]:
                continue
            zout = res.results[r]["zout"].astype(np.float32)
            for slot, t in enumerate(batch[r]):
                for c in range(C):
                    out[t, c, :] = (zout[4 * slot + c]
                                    + 1j * zout[64 + 4 * slot + c])
        if _trace:
            kernel._last_results = res
    return out


# revision 14
# speedup vs baseline: 1.6049x; 1.0551x over previous
"""Trainium2 Bass kernel for nn_SubspaceLinopFactory (subspace NUDFT forward).

Math (reference):
  s[a,c,h,w] = x[a,h,w] * mps[c,h,w]
  y[a,c,k]   = sum_hw s * exp(-i*(ty_k*gy_h + tx_k*gx_w))   (separable NUDFT)
  z[t,c,k]   = sum_a phi[a,t] * y[a,c,k] * sqrt_dcf[k],  r = subsamp_idx[t]
Sharding: trajectory r -> core r (R == 8 == n_cores).

Device design (v3, per core):
  gy pairing: gy[h]=h-32; conjugate pairs (+g,-g), g=1..31, halve the
  k-elementwise work; gy=0 joins the plus block; gy=-32 is a small residual
  unit. Host stages paired image columns (spm), phase fractions (range-
  reduced phases in turns), dcf, and phi-combined +-selector weights.
  Pipeline per (unit u, k-chunk) with triple-buffered PSUM [128,2,512]:
    stage 1 (TensorE fp16, 64x64 quadrant-tiled, 4 concurrent matmuls):
        RE = [Pp(plus@cos) rows 0-63 | Qm(minus@sin) rows 64-127]
        IM = [Qp(plus@sin)           | Pm(minus@cos)]
    product: prod = (ytab*dcf) (*) bank  -- DVE direct from PSUM, or
        ScalarE fp16 cast + DVE/GpSimd 2x, per-unit mode
    reduce (TensorE, col-tiled 128x64 pair): z_re += selphi_re.T @ prod_re,
        z_im += selphi_im.T @ prod_im  (phi + h-reduction in one matmul,
        accumulated over all units in one PSUM super)
  Trig tables on device: ScalarE Sin(2*pi*frac) -> fp16.
  Output: one PSUM->SBUF fp16 copy + DMA of z [128, K]; host scatters rows
  (t-slot, c) into [T, C, K] complex64.
"""
import numpy as np

A, T, C, R, D, K, H, W = 3, 32, 4, 8, 2, 1024, 64, 64
N_CORES = 8
AC = A * C           # 12
MT = AC // 2         # 6 m-tiles
NU = MT + 1          # units incl. residual
NSLOT = 16           # t-slots per launch (M = 4*NSLOT = 64)
KC = 512

# per-unit product mode: 'V' direct DVE from PSUM, 'SV' ScalarE cast + DVE,
# 'SG' ScalarE cast + GpSimd product. Resid is unit 6.
UNIT_MODE = ['V', 'V', 'V', 'V', 'SV', 'SG', 'SV']

_CACHE = {}


def _build_nc():
    import concourse.bacc as bacc
    import concourse.tile as tile
    import concourse.mybir as mybir

    AF = mybir.ActivationFunctionType
    OP = mybir.AluOpType
    F32 = mybir.dt.float32
    F16 = mybir.dt.float16
    TWO_PI = float(2 * np.pi)
    HALF_PI = float(np.pi / 2)

    nc = bacc.Bacc(None, target_bir_lowering=False)

    # consolidated inputs: xf = single-phase x fracs (cos via Sin bias);
    # ss = spm | selphi ; yd = y-main fracs | dcf ; yfr = resid fracs
    d_xf = nc.dram_tensor("xf", [128, K], F16, kind="ExternalInput")
    d_ss = nc.dram_tensor("ss", [128, 792 + 896], F16, kind="ExternalInput")
    d_yd = nc.dram_tensor("yd", [128, 2 * K], F16, kind="ExternalInput")
    d_yfr = nc.dram_tensor("yfr", [128, K], F16, kind="ExternalInput")
    d_zout = nc.dram_tensor("zout", [128, K], F16, kind="ExternalOutput")

    with tile.TileContext(nc) as tc:
        with (
            tc.tile_pool(name="cst", bufs=1) as cst,
            tc.tile_pool(name="work", bufs=3) as work,
            tc.tile_pool(name="cwork", bufs=2) as cwork,
            tc.tile_pool(name="psS", bufs=3, space="PSUM") as psS,
            tc.tile_pool(name="psZ", bufs=1, space="PSUM") as psZ,
        ):
            # PE warm-up: dense junk matmuls on a never-written scratch tile
            # trip the HAM activity monitor to full clock while DMAs land.
            scratch = cst.tile([128, KC], F16)
            nc.vector.memzero(scratch[:])
            bias_hp = cst.tile([128, 1], F32)
            nc.gpsimd.memset(bias_hp[:], HALF_PI)
            z = psZ.tile([128, K], F32)       # rows 0-63 z_re, 64-127 z_im
            for _ in range(10):
                nc.tensor.matmul(z[:, 0:KC], scratch[:, 0:128], scratch[:],
                                 start=True, stop=True, skip_group_check=True)

            xfrac = cst.tile([128, 2, KC], F16)    # [128, chunk, KC]
            spsel = cst.tile([128, 792 + 896], F16)
            ydm = cst.tile([128, 2, K], F16)       # [:,0,:]=yfm, [:,1,:]=dcf
            yfrac_r = cst.tile([128, K], F16)
            nc.sync.dma_start(xfrac[:], d_xf[:].rearrange(
                "p (c k) -> p c k", c=2))
            nc.gpsimd.dma_start(spsel[:], d_ss[:])
            nc.sync.dma_start(ydm[:], d_yd[:].rearrange(
                "p (c k) -> p c k", c=2))
            nc.gpsimd.dma_start(yfrac_r[:], d_yfr[:])
            spm = spsel[:, 0:792]
            selphi = spsel[:, 792:792 + 896]
            dcfb = ydm[:, 1, :]

            # trig tables: fp16 sin/cos via Sin(2*pi*frac [+ pi/2])
            xtab = cst.tile([128, 2, 2, KC], F16)  # [128, chunk, sin|cos, KC]
            ytab = cst.tile([128, 2, K], F16)   # [:,0,:]=main, [:,1,:]=resid
            # ytmd [128, 4, KC] = [Md-c0, Md-c0, Md-c1, Md-c1]
            ytmd = cst.tile([128, 4, KC], F16)
            ytrd = cst.tile([128, K], F16)
            zout_sb = cst.tile([128, K], F16)

            for kc in range(2):
                ks = slice(kc * KC, (kc + 1) * KC)
                nc.scalar.activation(xtab[:, kc, 0, :], xfrac[:, kc, :],
                                     AF.Sin, scale=TWO_PI)
                nc.scalar.activation(xtab[:, kc, 1, :], xfrac[:, kc, :],
                                     AF.Sin, scale=TWO_PI, bias=bias_hp[:])
                nc.scalar.activation(ytab[:, 0, ks], ydm[:, 0, ks],
                                     AF.Sin, scale=TWO_PI)
                for half in range(2):
                    nc.vector.tensor_tensor(ytmd[:, 2 * kc + half, :],
                                            ytab[:, 0, ks], dcfb[:, ks],
                                            OP.mult)
                nc.scalar.activation(ytab[:, 1, ks], yfrac_r[:, ks],
                                     AF.Sin, scale=TWO_PI)
                nc.vector.tensor_tensor(ytrd[:, ks], ytab[:, 1, ks],
                                        dcfb[:, ks], OP.mult)

                for u in range(NU):
                    mode = UNIT_MODE[u]
                    # PSUM per unit-chunk: [128, RE|IM, KC]
                    bank = psS.tile([128, 2, KC], F32, tag="bank")
                    cosx = xtab[0:64, kc, 1, :]
                    sinx = xtab[64:128, kc, 0, :]
                    if u < MT:
                        cb = u * 128
                        # 4 quadrant matmuls (auto tile_position from bases)
                        nc.tensor.matmul(bank[0:64, 0, :],
                                         spm[0:64, cb:cb + 64], cosx,
                                         start=True, stop=True)          # Pp
                        nc.tensor.matmul(bank[64:128, 0, :],
                                         spm[64:128, cb + 64:cb + 128], sinx,
                                         start=True, stop=True)          # Qm
                        nc.tensor.matmul(bank[0:64, 1, :],
                                         spm[64:128, cb:cb + 64], sinx,
                                         start=True, stop=True)          # Qp
                        nc.tensor.matmul(bank[64:128, 1, :],
                                         spm[0:64, cb + 64:cb + 128], cosx,
                                         start=True, stop=True)          # Pm
                        ncols, tsl = 2, ytmd[:, 2 * kc:2 * kc + 2, :]
                        bsl = bank[:, 0:2, :]
                    else:
                        nc.tensor.matmul(bank[0:24, 0, :],
                                         spm[0:64, 768:792], cosx,
                                         start=True, stop=True)     # P0 rows
                        nc.tensor.matmul(bank[64:88, 0, :],
                                         spm[64:128, 768:792], sinx,
                                         start=True, stop=True)     # Q0 rows
                        ncols, tsl = 1, ytrd[:, ks]
                        bsl = bank[:, 0, :]

                    prod = work.tile([128, 2, KC], F16, tag="prod")
                    psl = prod[:, 0:2, :] if ncols == 2 else prod[:, 0, :]
                    if mode == 'V':
                        nc.vector.tensor_tensor(psl, bsl, tsl, OP.mult)
                    else:
                        cast = cwork.tile([128, 2, KC], F16, tag="cast")
                        csl = cast[:, 0:2, :] if ncols == 2 else cast[:, 0, :]
                        nc.scalar.copy(csl, bsl)
                        if mode == 'SV':
                            nc.vector.tensor_tensor(psl, csl, tsl, OP.mult)
                        else:
                            nc.gpsimd.tensor_tensor(psl, csl, tsl, OP.mult)

                    # fused h-reduce + phi matmuls (col-tiled pair)
                    st = (u == 0)
                    sp = (u == NU - 1)
                    re_rhs = prod[:, 0, :]
                    im_rhs = prod[:, 1, :] if u < MT else prod[:, 0, :]
                    nc.tensor.matmul(z[0:64, ks],
                                     selphi[:, (2 * u) * 64:(2 * u + 1) * 64],
                                     re_rhs, start=st, stop=sp,
                                     skip_group_check=True)
                    nc.tensor.matmul(z[64:128, ks],
                                     selphi[:, (2 * u + 1) * 64:(2 * u + 2) * 64],
                                     im_rhs, start=st, stop=sp,
                                     skip_group_check=True)

                # per-chunk output: copy + DMA overlap the next chunk
                nc.scalar.copy(zout_sb[:, ks], z[:, ks])
                nc.gpsimd.dma_start(d_zout[:, ks], zout_sb[:, ks])

    nc.finalize()
    return nc


def _get_nc():
    if "nc" not in _CACHE:
        _CACHE["nc"] = _build_nc()
    return _CACHE["nc"]


def _stage_core(r, x, trj, phi, mps, sqrt_dcf, owned_ts):
    """Host staging for core r: layout/pairing of inputs, phase fractions
    (range-reduced phases in turns), and phi-signed selector weights."""
    f16 = np.float16
    ty = trj[r, 0, :].astype(np.float64)
    tx = trj[r, 1, :].astype(np.float64)
    inv2pi = 1.0 / (2 * np.pi)

    def frac(v):
        return v - np.round(v)

    gx = (np.arange(W) - W // 2).astype(np.float64)
    mx = np.outer(gx, tx) * inv2pi
    # chunk-major single-phase (cos comes from Sin's +pi/2 bias on device)
    xf = np.empty((128, 2, KC), np.float64)
    for kc in range(2):
        xf[:64, kc, :] = frac(mx[:, kc * KC:(kc + 1) * KC])
    xf[64:] = xf[:64]

    g = np.arange(32).astype(np.float64)
    my = np.outer(g, ty) * inv2pi
    yf = np.zeros((128, 2, K), np.float64)
    yf[0:32, 0, :] = frac(my + 0.25)
    yf[32:64, 0, :] = yf[0:32, 0, :]
    yf[64:96, 0, :] = frac(my)
    yf[96:128, 0, :] = yf[64:96, 0, :]
    m32 = 32.0 * ty * inv2pi
    yf[0:12, 1, :] = frac(m32 + 0.25)
    yf[12:24, 1, :] = frac(m32)
    yf[64:76, 1, :] = frac(m32)
    yf[76:88, 1, :] = frac(m32 + 0.25)

    dcfb = np.broadcast_to(sqrt_dcf[r].astype(f16), (128, K))

    s = (x[:, None, :, :] * mps[None, :, :, :]).reshape(AC, H, W)
    sp = s[:, 33:64, :]                     # gy = +1..+31
    sm = s[:, 31:0:-1, :]                   # gy = -1..-31
    spl = np.zeros((128, 792), np.float64)
    for j in range(MT):
        for i, ac in enumerate((2 * j, 2 * j + 1)):
            pc = j * 128 + i * 32
            mc = j * 128 + 64 + i * 32
            spl[:64, pc] = s[ac, 32, :]
            spl[:64, pc + 1:pc + 32] = (sp[ac] + sm[ac]).T
            spl[:64, mc + 1:mc + 32] = (sp[ac] - sm[ac]).T
    spl[:64, 768:780] = s[:, 0, :].T
    spl[:64, 780:792] = s[:, 0, :].T
    spl[64:] = spl[:64]

    selphi = np.zeros((128, 14 * 64), np.float64)
    for j in range(MT):
        sre = selphi[:, (2 * j) * 64:(2 * j) * 64 + 64]
        sim = selphi[:, (2 * j + 1) * 64:(2 * j + 1) * 64 + 64]
        for i, ac in enumerate((2 * j, 2 * j + 1)):
            a, c = divmod(ac, C)
            for slot, t in enumerate(owned_ts):
                m = 4 * slot + c
                p = phi[a, t]
                sre[i * 32:(i + 1) * 32, m] = p
                sre[64 + i * 32:96 + i * 32, m] = -p
                sim[i * 32:(i + 1) * 32, m] = -p
                sim[64 + i * 32:96 + i * 32, m] = -p
    sre = selphi[:, 12 * 64:13 * 64]
    sim = selphi[:, 13 * 64:14 * 64]
    for ac in range(AC):
        a, c = divmod(ac, C)
        for slot, t in enumerate(owned_ts):
            m = 4 * slot + c
            p = phi[a, t]
            sre[ac, m] = p
            sre[64 + ac, m] = p
            sim[12 + ac, m] = p
            sim[76 + ac, m] = -p

    return {
        "xf": xf.reshape(128, K).astype(f16),
        "ss": np.concatenate([spl, selphi], axis=1).astype(f16),
        "yd": np.concatenate([yf[:, 0, :],
                              dcfb.astype(np.float64)], axis=1).astype(f16),
        "yfr": yf[:, 1, :].astype(f16),
    }


def kernel(x, trj, phi, mps, sqrt_dcf, subsamp_idx, _trace=False):
    from concourse.bass_utils import run_bass_kernel_spmd

    x = np.asarray(x, dtype=np.float32)
    trj = np.asarray(trj, dtype=np.float32)
    phi = np.asarray(phi, dtype=np.float32)
    mps = np.asarray(mps, dtype=np.float32)
    sqrt_dcf = np.asarray(sqrt_dcf, dtype=np.float32)
    idx = np.asarray(subsamp_idx).astype(np.int64)

    nc = _get_nc()
    owned = {r: [t for t in range(T) if idx[t] == r] for r in range(R)}
    out = np.empty((T, C, K), dtype=np.complex64)
    launches = max(1, max((len(v) + NSLOT - 1) // NSLOT
                          for v in owned.values()))
    for li in range(launches):
        batch = {r: owned[r][li * NSLOT:(li + 1) * NSLOT] for r in range(R)}
        in_maps = [
            _stage_core(r, x, trj, phi, mps, sqrt_dcf, batch[r])
            for r in range(N_CORES)
        ]
        res = run_bass_kernel_spmd(nc, in_maps, core_ids=list(range(N_CORES)),
                                   trace=_trace)
        for r in range(N_CORES):
            if not batch[r]:
                continue
            zout = res.results[r]["zout"].astype(np.float32)
            for slot, t in enumerate(batch[r]):
                for c in range(C):
                    out[t, c, :] = (zout[4 * slot + c]
                                    + 1j * zout[64 + 4 * slot + c])
        if _trace:
            kernel._last_results = res
    return out
